# revision 10
# baseline (speedup 1.0000x reference)
"""Trainium2 Bass kernel for batched cross-attention (CoupletsAttentionModel).

Reference computation (per batch element b):
    S = dec @ enc^T          [S_dec, S_enc]
    P = softmax(S, axis=-1)
    O = P @ enc              [S_dec, D]

Sharding: data-parallel over batch — B=8 batch elements, one per NeuronCore.
Each core runs an identical (SPMD) program on its own batch slice; no
collectives, host stacks the 8 per-core outputs.

Per-core algorithm (S_enc=S_dec=2048, D=512, fp32 in/out), v3:
  - fp16 matmuls (4x faster than fp32; score err ~7e-3 rms -> out rel err
    ~2e-3 which passes the 2e-2 gate with 10x margin).
  - Input prep: load fp32, cast to f16 on VectorE, build enc^T / dec^T via
    PE transposes batched 4-to-a-PSUM-bank so each PSUM->SBUF copy moves
    [128,512] in one VectorE op.
  - Per 128-row q-tile: S row-block as 2x [128,1024] fp32 PSUM tiles
    (16 fp16 matmuls, k-chunk-major so softmax starts at 50% of the block),
    row-max on VectorE, exp(S-max) + row-sum fused on ScalarE (accum_out)
    writing fp16 P, P^T via ONE xbar transpose-DMA ([128,2048] ->
    [128,16,128]), P^T @ V accumulated in PSUM, 1/sum scale on VectorE.
"""

import contextlib
import ctypes
import os
import sys
import types

import numpy as np

import concourse.bass as bass
import concourse.tile as tile
from concourse import bacc, mybir
from concourse import bass_utils
from concourse.masks import make_identity

F32 = mybir.dt.float32
F16 = mybir.dt.float16
AX = mybir.AxisListType
AFT = mybir.ActivationFunctionType

N_CORES = 8
PART = 128


def attention_tile_kernel(tc, out_ap, dec_ap, enc_ap, seq, d):
    nc = tc.nc
    P = PART
    KC = 512  # matmul moving free dim / one fp32 PSUM bank
    SC = min(1024, seq)  # softmax chunk
    n_qt = seq // P
    n_kt = seq // P
    n_dt = d // P
    n_sc = seq // SC
    kc_per_sc = SC // KC

    stack = contextlib.ExitStack()
    pool = lambda **kw: stack.enter_context(tc.tile_pool(**kw))

    singles = pool(name="singles", bufs=1)
    big = pool(name="big", bufs=1)
    stage = pool(name="stage", bufs=3)
    stage16 = pool(name="stage16", bufs=3)
    psum = pool(name="psum", bufs=1, space="PSUM")
    p_pool = pool(name="p_pool", bufs=2)
    pt_pool = pool(name="pt_pool", bufs=2)
    stats = pool(name="stats", bufs=4)
    osb = pool(name="osb", bufs=2)

    with stack:
        ident = singles.tile([P, P], F16)
        make_identity(nc, ident[:])

        v_sb = big.tile([P, n_kt, d], F16)  # enc natural (V)
        # enc^T / dec^T as fine-grained tiles so matmul1 deps are range-level
        kT = [
            big.tile([P, n_dt, KC], F16, tag=f"kT_{kg}", name=f"kT_{kg}")
            for kg in range(seq // KC)
        ]
        qT = [
            big.tile([P, n_dt, P], F16, tag=f"qT_{qg}", name=f"qT_{qg}")
            for qg in range(n_qt)
        ]

        def prep_transposed(src16, dst, name):
            # 4 PE transposes into one PSUM bank, one [128, 512] copy out.
            # Shares PSUM slots with the s_ch tag (same byte size).
            tps_raw = psum.tile([P, SC * 2], F16, tag="s_ch", bufs=3, name=f"tps_{name}")
            tps = tps_raw[:, : n_dt * P].rearrange("p (a b) -> p a b", b=P)
            for dc in range(n_dt):
                nc.tensor.transpose(
                    tps[:, dc, :], src16[:, dc * P : (dc + 1) * P], ident[:]
                )
            nc.vector.tensor_copy(dst, tps[:])

        def prep_enc(st):
            e32 = stage.tile([P, d], F32, tag="ld32", name=f"e32_{st}")
            nc.sync.dma_start(out=e32[:], in_=enc_ap[st * P : (st + 1) * P, :])
            nc.vector.tensor_copy(v_sb[:, st, :], e32[:])
            kg, o = st // (KC // P), (st % (KC // P)) * P
            prep_transposed(v_sb[:, st, :], kT[kg][:, :, o : o + P], f"e_{st}")

        def prep_dec(st):
            d32 = stage.tile([P, d], F32, tag="ld32", name=f"d32_{st}")
            nc.sync.dma_start(out=d32[:], in_=dec_ap[st * P : (st + 1) * P, :])
            d16 = stage16.tile([P, d], F16, name=f"d16_{st}")
            nc.vector.tensor_copy(d16[:], d32[:])
            prep_transposed(d16[:], qT[st][:, :, :], f"d_{st}")

        # dec q-tile 0 first so matmul1 can start as soon as the first
        # enc k-range is transposed; the rest of dec streams in behind enc.
        prep_dec(0)
        for st in range(n_kt):
            prep_enc(st)
        for st in range(1, n_qt):
            prep_dec(st)

        # ---- main loop over q tiles ----
        for qt in range(n_qt):
            q0 = qt * P

            s_chunks = [
                psum.tile([P, SC], F32, tag="s_ch", bufs=3, name=f"s_ch_{qt}_{i}")
                for i in range(n_sc)
            ]
            for sc in range(n_sc):
                for j in range(kc_per_sc):
                    kg = sc * kc_per_sc + j
                    for dt_ in range(n_dt):
                        nc.tensor.matmul(
                            s_chunks[sc][:, j * KC : (j + 1) * KC],
                            qT[qt][:, dt_, :],
                            kT[kg][:, dt_, :],
                            start=(dt_ == 0),
                            stop=(dt_ == n_dt - 1),
                        )

            # row max (negated for exp bias)
            mx = stats.tile([P, n_sc], F32, tag="mx")
            for sc in range(n_sc):
                nc.vector.reduce_max(mx[:, sc : sc + 1], s_chunks[sc][:], axis=AX.X)
            negm = stats.tile([P, 1], F32, tag="negm")
            nc.vector.tensor_reduce(
                negm[:], mx[:], axis=AX.X, op=mybir.AluOpType.max, negate=True
            )

            # P = exp(S - max) f16 + fused row-sums
            p_sb = p_pool.tile([P, seq], F16)
            sums = stats.tile([P, n_sc], F32, tag="sums")
            for sc in range(n_sc):
                nc.scalar.activation(
                    p_sb[:, sc * SC : (sc + 1) * SC],
                    s_chunks[sc][:],
                    AFT.Exp,
                    bias=negm[:],
                    scale=1.0,
                    accum_out=sums[:, sc : sc + 1],
                )
            sm = stats.tile([P, 1], F32, tag="sm")
            nc.vector.reduce_sum(sm[:], sums[:], axis=AX.X)
            rinv = stats.tile([P, 1], F32, tag="rinv")
            nc.vector.reciprocal(rinv[:], sm[:])

            # P^T via xbar transpose-DMA: [128, seq] -> [128, n_kt, 128],
            # split in halves on two HWDGE queues (overlaps exp of 2nd half)
            pT3 = pt_pool.tile([P, n_kt, P], F16)
            half = seq // 2
            nc.scalar.dma_start(
                out=pT3[:, : n_kt // 2, :], in_=p_sb[:, :half], transpose=True
            )
            nc.sync.dma_start(
                out=pT3[:, n_kt // 2 :, :], in_=p_sb[:, half:], transpose=True
            )

            # O = P @ V accumulated over k tiles
            o_ch = psum.tile([P, d], F32, tag="o_ch", bufs=2, name=f"o_ch_{qt}")
            for kt in range(n_kt):
                nc.tensor.matmul(
                    o_ch[:],
                    pT3[:, kt, :],
                    v_sb[:, kt, :],
                    start=(kt == 0),
                    stop=(kt == n_kt - 1),
                )

            o_sb = osb.tile([P, d], F32)
            nc.vector.tensor_scalar_mul(o_sb[:], o_ch[:], rinv[:])
            nc.sync.dma_start(out=out_ap[q0 : q0 + P, :], in_=o_sb[:])


def build(seq=2048, d=512, n_cores=N_CORES):
    nc = bacc.Bacc(
        "TRN2", target_bir_lowering=False, debug=False, num_devices=n_cores
    )
    dec = nc.dram_tensor("dec", [seq, d], F32, kind="ExternalInput").ap()
    enc = nc.dram_tensor("enc", [seq, d], F32, kind="ExternalInput").ap()
    out = nc.dram_tensor("out", [seq, d], F32, kind="ExternalOutput").ap()
    with tile.TileContext(nc) as tc:
        attention_tile_kernel(tc, out, dec, enc, seq, d)
    nc.compile()
    return nc


# ---------------------------------------------------------------------------
# Optional NTFF profiling support (used by our own test harness; inert unless
# BASSKERNEL_TRACE=1). The agent image lacks `antenv.axon_hooks`, so recreate
# it in sys.modules with a ctypes hook against libaxon_pjrt.so.
# ---------------------------------------------------------------------------
LAST_EXEC_TIME_NS = None


def _install_profile_hook():
    so_path = "/opt/axon/libaxon_pjrt.so"
    if "antenv.axon_hooks" in sys.modules or not os.path.exists(so_path):
        return
    lib = ctypes.CDLL(so_path)
    if not hasattr(lib, "axon_start_nrt_profile"):
        return
    lib.axon_start_nrt_profile.argtypes = [
        ctypes.POINTER(ctypes.c_int64),
        ctypes.c_size_t,
    ]
    lib.axon_start_nrt_profile.restype = ctypes.c_int64
    lib.axon_stop_nrt_profile.argtypes = [ctypes.c_char_p]
    lib.axon_stop_nrt_profile.restype = ctypes.c_int64

    @contextlib.contextmanager
    def _hook(output_dir, device_ids):
        import jax

        jax.devices()
        if device_ids:
            ids = (ctypes.c_int64 * len(device_ids))(*device_ids)
            rc = lib.axon_start_nrt_profile(ids, len(device_ids))
        else:
            rc = lib.axon_start_nrt_profile(None, 0)
        if rc != 0:
            raise RuntimeError(f"axon_start_nrt_profile rc={rc}")
        try:
            yield
        finally:
            n = lib.axon_stop_nrt_profile(str(output_dir).encode())
            print(f"ntff profile: {n} file(s) written to {output_dir}")

    mod = types.ModuleType("antenv.axon_hooks")
    _state = {"hook": _hook}
    mod.set_axon_ntff_profile_hook = lambda h: _state.__setitem__("hook", h)
    mod.get_axon_ntff_profile_hook = lambda: _state["hook"]
    sys.modules["antenv.axon_hooks"] = mod
    bass_utils.upload_artifacts = lambda tmpdir: tmpdir


_NC_CACHE = {}


def kernel(enc_outputs: np.ndarray, dec_outputs: np.ndarray) -> np.ndarray:
    B, seq, d = dec_outputs.shape
    assert enc_outputs.shape == (B, seq, d) and B == N_CORES

    trace = os.environ.get("BASSKERNEL_TRACE", "0") == "1"
    if trace:
        _install_profile_hook()

    key = (seq, d)
    if key not in _NC_CACHE:
        _NC_CACHE[key] = build(seq, d)
    nc = _NC_CACHE[key]

    in_maps = [
        {
            "dec": np.ascontiguousarray(dec_outputs[b], dtype=np.float32),
            "enc": np.ascontiguousarray(enc_outputs[b], dtype=np.float32),
        }
        for b in range(B)
    ]
    res = bass_utils.run_bass_kernel_spmd(
        nc,
        in_maps,
        core_ids=list(range(N_CORES)),
        trace=trace,
        tmpdir=os.environ.get("BASSKERNEL_TRACE_DIR") if trace else None,
    )
    global LAST_EXEC_TIME_NS
    LAST_EXEC_TIME_NS = res.exec_time_ns
    out = np.stack([res.results[b]["out"] for b in range(B)], axis=0)
    return out.astype(np.float32)


# revision 23
# speedup vs baseline: 1.0249x; 1.0249x over previous
"""Trainium2 Bass kernel for batched cross-attention (CoupletsAttentionModel).

Reference computation (per batch element b):
    S = dec @ enc^T          [S_dec, S_enc]
    P = softmax(S, axis=-1)
    O = P @ enc              [S_dec, D]

Sharding: data-parallel over batch — B=8 batch elements, one per NeuronCore.
Each core runs an identical (SPMD) program on its own batch slice; no
collectives, host stacks the 8 per-core outputs.

Per-core algorithm (S_enc=S_dec=2048, D=512, fp32 in/out), v3:
  - fp16 matmuls (4x faster than fp32; score err ~7e-3 rms -> out rel err
    ~2e-3 which passes the 2e-2 gate with 10x margin).
  - Input prep: load fp32, cast to f16 on VectorE, build enc^T / dec^T via
    PE transposes batched 4-to-a-PSUM-bank so each PSUM->SBUF copy moves
    [128,512] in one VectorE op.
  - Per 128-row q-tile: S row-block as 2x [128,1024] fp32 PSUM tiles
    (16 fp16 matmuls, k-chunk-major so softmax starts at 50% of the block),
    row-max on VectorE, exp(S-max) + row-sum fused on ScalarE (accum_out)
    writing fp16 P, P^T via ONE xbar transpose-DMA ([128,2048] ->
    [128,16,128]), P^T @ V accumulated in PSUM, 1/sum scale on VectorE.
"""

import contextlib
import ctypes
import os
import sys
import types

import numpy as np

import concourse.bass as bass
import concourse.tile as tile
from concourse import bacc, mybir
from concourse import bass_utils
from concourse.masks import make_identity

F32 = mybir.dt.float32
F16 = mybir.dt.float16
AX = mybir.AxisListType
AFT = mybir.ActivationFunctionType

N_CORES = 8
PART = 128


def attention_tile_kernel(tc, out_ap, dec_ap, enc_ap, seq, d):
    nc = tc.nc
    P = PART
    KC = 512  # matmul moving free dim / one fp32 PSUM bank
    SC = min(1024, seq)  # softmax chunk
    n_qt = seq // P
    n_kt = seq // P
    n_dt = d // P
    n_sc = seq // SC
    kc_per_sc = SC // KC

    stack = contextlib.ExitStack()
    pool = lambda **kw: stack.enter_context(tc.tile_pool(**kw))

    singles = pool(name="singles", bufs=1)
    big = pool(name="big", bufs=1)
    stage = pool(name="stage", bufs=3)
    stage16 = pool(name="stage16", bufs=3)
    psum = pool(name="psum", bufs=1, space="PSUM")
    p_pool = pool(name="p_pool", bufs=3)
    pt_pool = pool(name="pt_pool", bufs=3)
    stats = pool(name="stats", bufs=4)
    osb = pool(name="osb", bufs=2)

    with stack:
        ident = singles.tile([P, P], F16)
        make_identity(nc, ident[:])

        v_sb = big.tile([P, n_kt, d], F16)  # enc natural (V)
        HG = seq if seq <= 1024 else seq // 2
        n_h = seq // HG
        kT = [
            big.tile([P, n_dt, HG], F16, tag=f"kT_{h}", name=f"kT_{h}")
            for h in range(n_h)
        ]
        qT = [
            big.tile([P, n_dt, HG], F16, tag=f"qT_{h}", name=f"qT_{h}")
            for h in range(n_h)
        ]

        def prep_transposed(src16, dst, name):
            # 4 PE transposes into one PSUM bank, one [128, 512] copy out.
            # Shares PSUM slots with the s_ch tag (same byte size).
            tps_raw = psum.tile([P, SC * 2], F16, tag="s_ch", bufs=3, name=f"tps_{name}")
            tps = tps_raw[:, : n_dt * P].rearrange("p (a b) -> p a b", b=P)
            for dc in range(n_dt):
                nc.tensor.transpose(
                    tps[:, dc, :], src16[:, dc * P : (dc + 1) * P], ident[:]
                )
            nc.vector.tensor_copy(dst, tps[:])

        def prep_enc(st):
            e32 = stage.tile([P, d], F32, tag="ld32", name=f"e32_{st}")
            (nc.sync if st % 2 == 0 else nc.scalar).dma_start(
                out=e32[:], in_=enc_ap[st * P : (st + 1) * P, :]
            )
            nc.scalar.copy(v_sb[:, st, :], e32[:])
            h, o = (st * P) // HG, (st * P) % HG
            prep_transposed(v_sb[:, st, :], kT[h][:, :, o : o + P], f"e_{st}")

        def prep_dec(st):
            d32 = stage.tile([P, d], F32, tag="ld32", name=f"d32_{st}")
            (nc.sync if st % 2 == 1 else nc.scalar).dma_start(
                out=d32[:], in_=dec_ap[st * P : (st + 1) * P, :]
            )
            d16 = stage16.tile([P, d], F16, name=f"d16_{st}")
            nc.vector.tensor_copy(d16[:], d32[:])
            h, o = (st * P) // HG, (st * P) % HG
            prep_transposed(d16[:], qT[h][:, :, o : o + P], f"d_{st}")

        # dec q-tile 0 first so matmul1 can start as soon as the first
        # enc k-range is transposed; the rest of dec streams in behind enc.
        prep_dec(0)
        for st in range(n_kt):
            prep_enc(st)
        for st in range(1, n_qt):
            prep_dec(st)

        # ---- main loop over q tiles ----
        for qt in range(n_qt):
            q0 = qt * P

            s_chunks = [
                psum.tile([P, SC], F32, tag="s_ch", bufs=3, name=f"s_ch_{qt}_{i}")
                for i in range(n_sc)
            ]
            for sc in range(n_sc):
                for j in range(kc_per_sc):
                    k0 = (sc * kc_per_sc + j) * KC
                    for dt_ in range(n_dt):
                        nc.tensor.matmul(
                            s_chunks[sc][:, j * KC : (j + 1) * KC],
                            qT[q0 // HG][:, dt_, q0 % HG : q0 % HG + P],
                            kT[k0 // HG][:, dt_, k0 % HG : k0 % HG + KC],
                            start=(dt_ == 0),
                            stop=(dt_ == n_dt - 1),
                        )

            # row max (negated for exp bias)
            mx = stats.tile([P, n_sc], F32, tag="mx")
            for sc in range(n_sc):
                nc.vector.reduce_max(mx[:, sc : sc + 1], s_chunks[sc][:], axis=AX.X)
            negm = stats.tile([P, 1], F32, tag="negm")
            nc.vector.tensor_reduce(
                negm[:], mx[:], axis=AX.X, op=mybir.AluOpType.max, negate=True
            )

            # P = exp(S - max) f16 + fused row-sums
            p_sb = p_pool.tile([P, seq], F16)
            sums = stats.tile([P, n_sc], F32, tag="sums")
            for sc in range(n_sc):
                nc.scalar.activation(
                    p_sb[:, sc * SC : (sc + 1) * SC],
                    s_chunks[sc][:],
                    AFT.Exp,
                    bias=negm[:],
                    scale=1.0,
                    accum_out=sums[:, sc : sc + 1],
                )
            sm = stats.tile([P, 1], F32, tag="sm")
            nc.vector.reduce_sum(sm[:], sums[:], axis=AX.X)
            rinv = stats.tile([P, 1], F32, tag="rinv")
            nc.vector.reciprocal(rinv[:], sm[:])

            # P^T via xbar transpose-DMA: [128, seq] -> [128, n_kt, 128],
            # split in halves on two HWDGE queues (overlaps exp of 2nd half)
            pT3 = pt_pool.tile([P, n_kt, P], F16)
            half = seq // 2
            nc.scalar.dma_start(
                out=pT3[:, : n_kt // 2, :], in_=p_sb[:, :half], transpose=True
            )
            nc.sync.dma_start(
                out=pT3[:, n_kt // 2 :, :], in_=p_sb[:, half:], transpose=True
            )

            # O = P @ V accumulated over k tiles
            o_ch = psum.tile([P, d], F32, tag="o_ch", bufs=2, name=f"o_ch_{qt}")
            for kt in range(n_kt):
                nc.tensor.matmul(
                    o_ch[:],
                    pT3[:, kt, :],
                    v_sb[:, kt, :],
                    start=(kt == 0),
                    stop=(kt == n_kt - 1),
                )

            o_sb = osb.tile([P, d], F32)
            nc.vector.tensor_scalar_mul(o_sb[:], o_ch[:], rinv[:])
            nc.sync.dma_start(out=out_ap[q0 : q0 + P, :], in_=o_sb[:])


def build(seq=2048, d=512, n_cores=N_CORES):
    nc = bacc.Bacc(
        "TRN2", target_bir_lowering=False, debug=False, num_devices=n_cores
    )
    dec = nc.dram_tensor("dec", [seq, d], F32, kind="ExternalInput").ap()
    enc = nc.dram_tensor("enc", [seq, d], F32, kind="ExternalInput").ap()
    out = nc.dram_tensor("out", [seq, d], F32, kind="ExternalOutput").ap()
    with tile.TileContext(nc) as tc:
        attention_tile_kernel(tc, out, dec, enc, seq, d)
    nc.compile()
    return nc


# ---------------------------------------------------------------------------
# Optional NTFF profiling support (used by our own test harness; inert unless
# BASSKERNEL_TRACE=1). The agent image lacks `antenv.axon_hooks`, so recreate
# it in sys.modules with a ctypes hook against libaxon_pjrt.so.
# ---------------------------------------------------------------------------
LAST_EXEC_TIME_NS = None


def _install_profile_hook():
    so_path = "/opt/axon/libaxon_pjrt.so"
    if "antenv.axon_hooks" in sys.modules or not os.path.exists(so_path):
        return
    lib = ctypes.CDLL(so_path)
    if not hasattr(lib, "axon_start_nrt_profile"):
        return
    lib.axon_start_nrt_profile.argtypes = [
        ctypes.POINTER(ctypes.c_int64),
        ctypes.c_size_t,
    ]
    lib.axon_start_nrt_profile.restype = ctypes.c_int64
    lib.axon_stop_nrt_profile.argtypes = [ctypes.c_char_p]
    lib.axon_stop_nrt_profile.restype = ctypes.c_int64

    @contextlib.contextmanager
    def _hook(output_dir, device_ids):
        import jax

        jax.devices()
        if device_ids:
            ids = (ctypes.c_int64 * len(device_ids))(*device_ids)
            rc = lib.axon_start_nrt_profile(ids, len(device_ids))
        else:
            rc = lib.axon_start_nrt_profile(None, 0)
        if rc != 0:
            raise RuntimeError(f"axon_start_nrt_profile rc={rc}")
        try:
            yield
        finally:
            n = lib.axon_stop_nrt_profile(str(output_dir).encode())
            print(f"ntff profile: {n} file(s) written to {output_dir}")

    mod = types.ModuleType("antenv.axon_hooks")
    _state = {"hook": _hook}
    mod.set_axon_ntff_profile_hook = lambda h: _state.__setitem__("hook", h)
    mod.get_axon_ntff_profile_hook = lambda: _state["hook"]
    sys.modules["antenv.axon_hooks"] = mod
    bass_utils.upload_artifacts = lambda tmpdir: tmpdir


_NC_CACHE = {}


def kernel(enc_outputs: np.ndarray, dec_outputs: np.ndarray) -> np.ndarray:
    B, seq, d = dec_outputs.shape
    assert enc_outputs.shape == (B, seq, d) and B == N_CORES

    trace = os.environ.get("BASSKERNEL_TRACE", "0") == "1"
    if trace:
        _install_profile_hook()

    key = (seq, d)
    if key not in _NC_CACHE:
        _NC_CACHE[key] = build(seq, d)
    nc = _NC_CACHE[key]

    in_maps = [
        {
            "dec": np.ascontiguousarray(dec_outputs[b], dtype=np.float32),
            "enc": np.ascontiguousarray(enc_outputs[b], dtype=np.float32),
        }
        for b in range(B)
    ]
    res = bass_utils.run_bass_kernel_spmd(
        nc,
        in_maps,
        core_ids=list(range(N_CORES)),
        trace=trace,
        tmpdir=os.environ.get("BASSKERNEL_TRACE_DIR") if trace else None,
    )
    global LAST_EXEC_TIME_NS
    LAST_EXEC_TIME_NS = res.exec_time_ns
    out = np.stack([res.results[b]["out"] for b in range(B)], axis=0)
    return out.astype(np.float32)
